# revision 7
# baseline (speedup 1.0000x reference)
"""Trainium2 Bass kernel for nn_AttentionBlock (GroupNorm + MHA + proj + residual).

Sharding: data-parallel over batch; 8 batches -> 8 NeuronCores, one batch each.

Per-core layout (c=512 channels, t=1024 spatial, H=8 heads, ch=64):
  - x, h kept as [c-on-partitions, t] (4 tiles of [128, 1024])
  - q,k computed as [o, t] with per-head tiles: tile hd = [q_hd(64 rows); k_hd(64 rows)]
    (w_qkv rows permuted host-side; q side pre-scaled by 1/sqrt(ch) = 0.125, exact)
  - v computed directly transposed: vT[t, o_v] via matmul(lhsT=h, rhs=WvT), with a
    ones column appended per head -> AV matmul also produces the softmax denominator Z
  - scores computed transposed: S^T[s, t] = k^T q  (matmul lhsT=k, rhs=q), so that
    exp(S^T) (ACT, psum->sbuf) feeds the AV matmul with s as the contraction dim
  - softmax skips the max-subtraction (scores are ~N(0,1); exp is safe in fp32)
  - a[c, t] accumulated over s-tiles; row 64 = Z; divide via reciprocal + DMA
    broadcast (DRAM round-trip) + tensor mult during psum evacuation
  - proj + bias + residual fused in one scalar_tensor_tensor per output tile
All matmuls run in fp32r (full PE rate at N=512; ~1.5e-4 max rel err vs fp64).
"""

import numpy as np

B, C, HW, T = 8, 512, 32, 1024
H, CH = 8, 64
G, GS = 32, 16
EPS = 1e-5
NCORES = 8

_CACHE = {}
TRACE = False  # test harness can set kernel.TRACE = True to get a profile


def _install_ntff_hook():
    import sys, types
    if 'antenv.axon_hooks' in sys.modules:
        return
    mod = types.ModuleType('antenv.axon_hooks')
    state = {'hook': None}
    mod.set_axon_ntff_profile_hook = lambda h: state.__setitem__('hook', h)
    mod.get_axon_ntff_profile_hook = lambda: state['hook']
    sys.modules['antenv.axon_hooks'] = mod
    import antenv
    antenv.axon_hooks = mod
    try:
        from trn_agent_boot.trn_boot import _ntff_profile_via_ctypes
        mod.set_axon_ntff_profile_hook(_ntff_profile_via_ctypes('/opt/axon/libaxon_pjrt.so'))
    except Exception:
        pass


def _split_multi_waits(nc, max_waits=1):
    """This container's walrus supports only one sync wait per instruction; move
    extra waits onto same-engine no-ops inserted just before the instruction."""
    import concourse.mybir as mybir
    for f in nc.m.functions:
        for bb in f.blocks:
            insts = bb.instructions
            out = []
            changed = False
            for inst in insts:
                si = inst.sync_info
                waits = list(si.on_wait) if si is not None and si.on_wait else []
                if len(waits) > max_waits:
                    changed = True
                    for j, w in enumerate(waits[:-max_waits]):
                        out.append(mybir.InstNoOp(
                            name=f"{inst.name}-ws{j}",
                            sync_info=mybir.SyncInfo(on_wait=[w], on_update=[]),
                            bass_nofuse=True,
                            engine=inst.engine,
                        ))
                    inst.sync_info = mybir.SyncInfo(
                        on_wait=waits[-max_waits:],
                        on_update=list(si.on_update) if si.on_update else [],
                    )
                out.append(inst)
            if changed:
                bb.instructions = out


def _build_nc():
    import concourse.bass as bass
    import concourse.tile as tile
    import concourse.mybir as mybir

    f32 = mybir.dt.float32
    f32r = mybir.dt.float32r
    Alu = mybir.AluOpType
    Act = mybir.ActivationFunctionType

    nc = bass.Bass()

    xin = nc.dram_tensor("xin", [C, T], f32, kind="ExternalInput")
    wqkT = nc.dram_tensor("wqkT", [C, 1024], f32r, kind="ExternalInput")   # [c, (hd,128)]
    wvT = nc.dram_tensor("wvT", [C, C], f32r, kind="ExternalInput")        # [c, (hd,64)]
    wpT = nc.dram_tensor("wpT", [C, C], f32r, kind="ExternalInput")        # [c(hd-major), o]
    bqk = nc.dram_tensor("bqk", [1024], f32, kind="ExternalInput")         # per (hd,128) row
    bv = nc.dram_tensor("bv", [C], f32, kind="ExternalInput")              # head-major v bias
    bp = nc.dram_tensor("bp", [C], f32, kind="ExternalInput")
    gam = nc.dram_tensor("gam", [C], f32, kind="ExternalInput")
    bet = nc.dram_tensor("bet", [C], f32, kind="ExternalInput")
    onesc = nc.dram_tensor("onesc", [128, 8], f32r, kind="ExternalInput")
    outd = nc.dram_tensor("outd", [C, T], f32, kind="ExternalOutput")

    with tile.TileContext(nc) as tc:
        with tc.tile_pool(name="const", bufs=1) as const, \
             tc.tile_pool(name="big", bufs=1) as big, \
             tc.tile_pool(name="qkp", bufs=4) as qkp, \
             tc.tile_pool(name="esp", bufs=8) as esp, \
             tc.tile_pool(name="zp", bufs=2) as zp, \
             tc.tile_pool(name="gn", bufs=2) as gn, \
             tc.tile_pool(name="ps", bufs=2, space="PSUM") as ps, \
             tc.tile_pool(name="dram", bufs=2, space="DRAM") as dram:

            # ---- constant / weight loads ----
            wqkT_t = const.tile([128, 4, 1024], f32r)
            nc.sync.dma_start(out=wqkT_t, in_=wqkT.rearrange("(ci p) o -> p ci o", p=128))
            wvT_t = const.tile([128, 4, 512], f32r)
            nc.sync.dma_start(out=wvT_t, in_=wvT.rearrange("(ci p) o -> p ci o", p=128))
            wpT_t = const.tile([128, 4, 512], f32r)
            nc.sync.dma_start(out=wpT_t, in_=wpT.rearrange("(ci p) o -> p ci o", p=128))
            bqk_t = const.tile([128, 8], f32)
            nc.sync.dma_start(out=bqk_t, in_=bqk.rearrange("(oi p) -> p oi", p=128))
            bv_b = const.tile([128, 512], f32)
            nc.sync.dma_start(out=bv_b, in_=bass.AP(
                tensor=bv, offset=0, ap=[[0, 128], [1, 512]]))
            bp_t = const.tile([128, 4], f32)
            nc.sync.dma_start(out=bp_t, in_=bp.rearrange("(ci p) -> p ci", p=128))
            gam_t = const.tile([128, 4], f32)
            nc.sync.dma_start(out=gam_t, in_=gam.rearrange("(ci p) -> p ci", p=128))
            bet_t = const.tile([128, 4], f32)
            nc.sync.dma_start(out=bet_t, in_=bet.rearrange("(ci p) -> p ci", p=128))
            eps_t = const.tile([128, 1], f32)
            nc.vector.memset(eps_t, EPS)

            xt = big.tile([128, 4, 1024], f32)
            nc.sync.dma_start(out=xt, in_=xin.rearrange("(ci p) t -> p ci t", p=128))

            # ---- GroupNorm ----
            # per-channel mean/var over t (bn_stats in 2 chunks of 512)
            chmv = gn.tile([128, 4, 2], f32)
            for ci in range(4):
                st = gn.tile([128, 2, 6], f32, tag="st")
                xv = xt[:, ci, :].rearrange("p (n f) -> p n f", f=512)
                for sub in range(2):
                    nc.vector.bn_stats(out=st[:, sub, :], in_=xv[:, sub, :])
                nc.vector.bn_aggr(out=chmv[:, ci, :], in_=st)
            # shuffle per-channel stats -> per-group rows [32, 16, 2]
            gst = gn.tile([32, 16, 2], f32)
            for ci in range(4):
                nc.sync.dma_start(out=gst[ci * 8:(ci + 1) * 8, :, :], in_=chmv[:, ci, :])
            # group mean / rstd  [32, 1]
            sm = gn.tile([32, 1], f32)
            nc.vector.tensor_reduce(out=sm, in_=gst[:, :, 0:1].rearrange("g j k -> g (j k)"),
                                    axis=mybir.AxisListType.X, op=Alu.add)
            s2 = gn.tile([32, 16], f32)
            nc.vector.tensor_mul(out=s2, in0=gst[:, :, 0:1].rearrange("g j k -> g (j k)"),
                                 in1=gst[:, :, 0:1].rearrange("g j k -> g (j k)"))
            nc.vector.tensor_add(out=s2, in0=s2, in1=gst[:, :, 1:2].rearrange("g j k -> g (j k)"))
            ss = gn.tile([32, 1], f32)
            nc.vector.tensor_reduce(out=ss, in_=s2, axis=mybir.AxisListType.X, op=Alu.add)
            grp = gn.tile([32, 2], f32)
            mg = grp[:, 0:1]
            nc.vector.tensor_scalar_mul(out=mg, in0=sm, scalar1=1.0 / GS)
            vg = grp[:, 1:2]
            nc.vector.tensor_scalar_mul(out=vg, in0=ss, scalar1=1.0 / GS)
            mg2 = gn.tile([32, 1], f32)
            nc.vector.tensor_mul(out=mg2, in0=mg, in1=mg)
            nc.vector.tensor_sub(out=vg, in0=vg, in1=mg2)
            # rstd = 1/sqrt(vg + eps)
            nc.scalar.activation(out=vg, in_=vg, func=Act.Sqrt, bias=eps_t[:32], scale=1.0)
            nc.vector.reciprocal(out=vg, in_=vg)
            # broadcast group stats back to channels via DRAM round-trip
            gd = dram.tile([32, 2], f32)
            nc.sync.dma_start(out=gd, in_=grp)
            chms = gn.tile([128, 4, 2], f32)
            for ci in range(4):
                src = bass.AP(tensor=gd.tensor, offset=gd.offset + ci * 8 * 2,
                              ap=[[2, 8], [0, 16], [1, 2]])
                nc.sync.dma_start(out=chms[:, ci, :], in_=src)
            # per-channel scale/shift  [128, 4]
            scl = gn.tile([128, 4], f32)
            nc.vector.tensor_mul(out=scl, in0=gam_t, in1=chms[:, :, 1])
            sht = gn.tile([128, 4], f32)
            nc.vector.tensor_mul(out=sht, in0=scl, in1=chms[:, :, 0])
            nc.vector.tensor_sub(out=sht, in0=bet_t, in1=sht)
            # h = x * scl + sht   (f32r, feeds matmuls)
            ht = big.tile([128, 4, 1024], f32r)
            for ci in range(4):
                nc.vector.tensor_scalar(out=ht[:, ci, :], in0=xt[:, ci, :],
                                        scalar1=scl[:, ci:ci + 1], scalar2=sht[:, ci:ci + 1],
                                        op0=Alu.mult, op1=Alu.add)

            # ---- q,k = WqkT.T @ h ----
            # o-blocks 0..3: q for head pairs (0,1)..(6,7); blocks 4..7: k likewise.
            # qk[pi] layout: [p, 2, t] with p<64 = head 2pi, p>=64 = head 2pi+1;
            # slot 0 = q, slot 1 = k  (so lhsT/rhs of the score matmul share a base
            # partition, and the two heads of a pair occupy different PE row groups).
            qk = [qkp.tile([128, 2, 1024], f32r, tag="qk", name=f"qk{i}") for i in range(4)]
            for oi in range(8):
                pi, side = oi % 4, oi // 4
                pqk = ps.tile([128, 1024], f32, tag="s")
                for ni in range(2):
                    for ci in range(4):
                        nc.tensor.matmul(pqk[:, ni * 512:(ni + 1) * 512],
                                         lhsT=wqkT_t[:, ci, oi * 128:(oi + 1) * 128],
                                         rhs=ht[:, ci, ni * 512:(ni + 1) * 512],
                                         start=(ci == 0), stop=(ci == 3))
                nc.vector.tensor_scalar_add(out=qk[pi][:, side, :], in0=pqk,
                                            scalar1=bqk_t[:, oi:oi + 1])

            # ---- vT[t, o_v] (+ ones col per head) ----
            vTa = big.tile([128, 8, 8, 65], f32r)  # [t_part, ti, hd, ch+1]
            for ti in range(8):
                nc.sync.dma_start(out=vTa[:, ti, :, 64:65].rearrange("p h k -> p (h k)"),
                                  in_=onesc[:, :])
                pv = ps.tile([128, 1024], f32, tag="s")
                for ci in range(4):
                    nc.tensor.matmul(pv[:, 0:512], lhsT=ht[:, ci, ti * 128:(ti + 1) * 128],
                                     rhs=wvT_t[:, ci, :], start=(ci == 0), stop=(ci == 3))
                nc.vector.tensor_add(
                    out=vTa[:, ti, :, 0:64],
                    in0=pv[:, 0:512].rearrange("p (h c) -> p h c", h=8),
                    in1=bv_b.rearrange("p (h c) -> p h c", h=8))

            # ---- attention, one head pair at a time ----
            at_ = big.tile([128, 4, 1024], f32r)  # a[c(hd-major), t]
            for pi in range(4):
                pa = [ps.tile([128, 1024], f32, tag="a", name=f"pa{pi}_{i}") for i in range(2)]
                for si in range(8):
                    pss = [ps.tile([128, 1024], f32, tag="s", name=f"pss{pi}_{si}_{i}") for i in range(2)]
                    # score matmuls for both heads: different PE row groups -> concurrent
                    for half in range(2):
                        base = 64 * half
                        for ni in range(2):
                            nc.tensor.matmul(
                                pss[half][:, ni * 512:(ni + 1) * 512],
                                lhsT=qk[pi][base:base + 64, 1, si * 128:(si + 1) * 128],
                                rhs=qk[pi][base:base + 64, 0, ni * 512:(ni + 1) * 512],
                                start=True, stop=True)
                    ess = []
                    for half in range(2):
                        es = esp.tile([128, 1024], f32r, tag="es")
                        nc.scalar.activation(out=es, in_=pss[half], func=Act.Exp)
                        ess.append(es)
                    for half in range(2):
                        hd = 2 * pi + half
                        for ni in range(2):
                            nc.tensor.matmul(pa[half][0:65, ni * 512:(ni + 1) * 512],
                                             lhsT=vTa[:, si, hd, :],
                                             rhs=ess[half][:, ni * 512:(ni + 1) * 512],
                                             start=(si == 0), stop=(si == 7))
                # softmax denominator: row 64 of pa
                for half in range(2):
                    hd = 2 * pi + half
                    zr = zp.tile([1, 1024], f32, tag="zr")
                    nc.vector.reciprocal(out=zr, in_=pa[half][64:65, :])
                    zd = dram.tile([1, 1024], f32, tag="zd")
                    nc.sync.dma_start(out=zd, in_=zr)
                    zb = zp.tile([64, 1024], f32, tag="zb")
                    nc.sync.dma_start(out=zb, in_=bass.AP(
                        tensor=zd.tensor, offset=zd.offset, ap=[[0, 64], [1, 1024]]))
                    nc.vector.tensor_tensor(
                        out=at_[(hd % 2) * 64:(hd % 2) * 64 + 64, hd // 2, :],
                        in0=pa[half][0:64, :], in1=zb, op=Alu.mult)

            # ---- proj + bias + residual ----
            for oi in range(4):
                pp = ps.tile([128, 1024], f32, tag="s")
                for ni in range(2):
                    for ci in range(4):
                        nc.tensor.matmul(pp[:, ni * 512:(ni + 1) * 512],
                                         lhsT=wpT_t[:, ci, oi * 128:(oi + 1) * 128],
                                         rhs=at_[:, ci, ni * 512:(ni + 1) * 512],
                                         start=(ci == 0), stop=(ci == 3))
                ot = qkp.tile([128, 1024], f32, tag="qk")
                nc.vector.scalar_tensor_tensor(out=ot, in0=pp, scalar=bp_t[:, oi:oi + 1],
                                               in1=xt[:, oi, :], op0=Alu.add, op1=Alu.add)
                nc.sync.dma_start(
                    out=outd.rearrange("(ci p) t -> p ci t", p=128)[:, oi, :], in_=ot)

    _split_multi_waits(nc)
    return nc


def _prep_host(x, gamma, beta, w_qkv, b_qkv, w_proj, b_proj):
    """Host-side weight permutation/pre-scaling + per-core input maps."""
    x = np.ascontiguousarray(x, dtype=np.float32).reshape(B, C, T)
    scale2 = 1.0 / np.sqrt(CH)  # folded into q (exact: 0.125 is a power of two)

    w = np.asarray(w_qkv, dtype=np.float32).reshape(H, 3, CH, C)
    bq = np.asarray(b_qkv, dtype=np.float32).reshape(H, 3, CH)
    wq = w[:, 0] * scale2          # [hd, 64, c]
    wk = w[:, 1]
    wv = w[:, 2]
    # o-block order: 4 q-blocks (one per head pair: [q_{2i}; q_{2i+1}]), 4 k-blocks
    qcols = wq.reshape(4, 128, C).transpose(2, 0, 1)            # [c, pi, 128]
    kcols = wk.reshape(4, 128, C).transpose(2, 0, 1)
    wqkT_host = np.ascontiguousarray(
        np.concatenate([qcols, kcols], axis=1).reshape(C, 1024))
    bqk_host = np.ascontiguousarray(np.concatenate(
        [(bq[:, 0] * scale2).reshape(4, 128), bq[:, 1].reshape(4, 128)], axis=0
    ).reshape(1024))
    wvT_host = np.ascontiguousarray(wv.transpose(2, 0, 1).reshape(C, C))
    bv_host = np.ascontiguousarray(bq[:, 2].reshape(C))
    wpT_host = np.ascontiguousarray(np.asarray(w_proj, dtype=np.float32).T)
    ones_host = np.ones((128, 8), np.float32)

    common = {
        "wqkT": wqkT_host, "wvT": wvT_host, "wpT": wpT_host,
        "bqk": bqk_host, "bv": bv_host,
        "bp": np.ascontiguousarray(np.asarray(b_proj, np.float32)),
        "gam": np.ascontiguousarray(np.asarray(gamma, np.float32)),
        "bet": np.ascontiguousarray(np.asarray(beta, np.float32)),
        "onesc": ones_host,
    }
    return [dict(common, xin=np.ascontiguousarray(x[b])) for b in range(B)]


def kernel(x, gamma, beta, w_qkv, b_qkv, w_proj, b_proj):
    from concourse.bass_utils import run_bass_kernel_spmd

    if "nc" not in _CACHE:
        _CACHE["nc"] = _build_nc()
    nc = _CACHE["nc"]

    in_maps = _prep_host(x, gamma, beta, w_qkv, b_qkv, w_proj, b_proj)
    kwargs = {}
    if TRACE:
        _install_ntff_hook()
        kwargs["trace"] = True
    res = run_bass_kernel_spmd(nc, in_maps, core_ids=list(range(NCORES)), **kwargs)
    if TRACE:
        _CACHE["last_result"] = res
    out = np.stack([r["outd"] for r in res.results], axis=0)
    return out.reshape(B, C, HW, HW)


# revision 12
# speedup vs baseline: 1.1424x; 1.1424x over previous
"""Trainium2 Bass kernel for nn_AttentionBlock (GroupNorm + MHA + proj + residual).

Sharding: data-parallel over batch; 8 batches -> 8 NeuronCores, one batch each.

Per-core layout (c=512 channels, t=1024 spatial, H=8 heads, ch=64):
  - x, h kept as [c-on-partitions, t] (4 tiles of [128, 1024])
  - q,k computed as [o, t] with per-head tiles: tile hd = [q_hd(64 rows); k_hd(64 rows)]
    (w_qkv rows permuted host-side; q side pre-scaled by 1/sqrt(ch) = 0.125, exact)
  - v computed directly transposed: vT[t, o_v] via matmul(lhsT=h, rhs=WvT), with a
    ones column appended per head -> AV matmul also produces the softmax denominator Z
  - scores computed transposed: S^T[s, t] = k^T q  (matmul lhsT=k, rhs=q), so that
    exp(S^T) (ACT, psum->sbuf) feeds the AV matmul with s as the contraction dim
  - softmax skips the max-subtraction (scores are ~N(0,1); exp is safe in fp32)
  - a[c, t] accumulated over s-tiles; row 64 = Z; divide via reciprocal + DMA
    broadcast (DRAM round-trip) + tensor mult during psum evacuation
  - proj + bias + residual fused in one scalar_tensor_tensor per output tile
All matmuls run in fp32r (full PE rate at N=512; ~1.5e-4 max rel err vs fp64).
"""

import numpy as np

B, C, HW, T = 8, 512, 32, 1024
H, CH = 8, 64
G, GS = 32, 16
EPS = 1e-5
NCORES = 8

_CACHE = {}
TRACE = False  # test harness can set kernel.TRACE = True to get a profile


def _install_ntff_hook():
    import sys, types
    if 'antenv.axon_hooks' in sys.modules:
        return
    mod = types.ModuleType('antenv.axon_hooks')
    state = {'hook': None}
    mod.set_axon_ntff_profile_hook = lambda h: state.__setitem__('hook', h)
    mod.get_axon_ntff_profile_hook = lambda: state['hook']
    sys.modules['antenv.axon_hooks'] = mod
    import antenv
    antenv.axon_hooks = mod
    try:
        from trn_agent_boot.trn_boot import _ntff_profile_via_ctypes
        mod.set_axon_ntff_profile_hook(_ntff_profile_via_ctypes('/opt/axon/libaxon_pjrt.so'))
    except Exception:
        pass


def _split_multi_waits(nc, max_waits=1):
    """This container's walrus supports only one sync wait per instruction; move
    extra waits onto same-engine no-ops inserted just before the instruction."""
    import concourse.mybir as mybir
    for f in nc.m.functions:
        for bb in f.blocks:
            insts = bb.instructions
            out = []
            changed = False
            for inst in insts:
                si = inst.sync_info
                waits = list(si.on_wait) if si is not None and si.on_wait else []
                if len(waits) > max_waits:
                    changed = True
                    for j, w in enumerate(waits[:-max_waits]):
                        out.append(mybir.InstNoOp(
                            name=f"{inst.name}-ws{j}",
                            sync_info=mybir.SyncInfo(on_wait=[w], on_update=[]),
                            bass_nofuse=True,
                            engine=inst.engine,
                        ))
                    inst.sync_info = mybir.SyncInfo(
                        on_wait=waits[-max_waits:],
                        on_update=list(si.on_update) if si.on_update else [],
                    )
                out.append(inst)
            if changed:
                bb.instructions = out


def _build_nc():
    import concourse.bass as bass
    import concourse.tile as tile
    import concourse.mybir as mybir

    f32 = mybir.dt.float32
    f32r = mybir.dt.float32r
    Alu = mybir.AluOpType
    Act = mybir.ActivationFunctionType

    nc = bass.Bass()

    xin = nc.dram_tensor("xin", [C, T], f32, kind="ExternalInput")
    wqkT = nc.dram_tensor("wqkT", [C, 1024], f32r, kind="ExternalInput")   # [c, (hd,128)]
    wvT = nc.dram_tensor("wvT", [C, C], f32r, kind="ExternalInput")        # [c, (hd,64)]
    wpT = nc.dram_tensor("wpT", [C, C], f32r, kind="ExternalInput")        # [c(hd-major), o]
    bqk = nc.dram_tensor("bqk", [1024], f32, kind="ExternalInput")         # per (hd,128) row
    bv = nc.dram_tensor("bv", [C], f32, kind="ExternalInput")              # head-major v bias
    bp = nc.dram_tensor("bp", [C], f32, kind="ExternalInput")
    gam = nc.dram_tensor("gam", [C], f32, kind="ExternalInput")
    bet = nc.dram_tensor("bet", [C], f32, kind="ExternalInput")
    onesc = nc.dram_tensor("onesc", [128, 8], f32r, kind="ExternalInput")
    outd = nc.dram_tensor("outd", [C, T], f32, kind="ExternalOutput")

    with tile.TileContext(nc) as tc:
        with tc.tile_pool(name="const", bufs=1) as const, \
             tc.tile_pool(name="big", bufs=1) as big, \
             tc.tile_pool(name="qkp", bufs=4) as qkp, \
             tc.tile_pool(name="esp", bufs=8) as esp, \
             tc.tile_pool(name="zp", bufs=2) as zp, \
             tc.tile_pool(name="gn", bufs=2) as gn, \
             tc.tile_pool(name="ps", bufs=2, space="PSUM") as ps, \
             tc.tile_pool(name="dram", bufs=2, space="DRAM") as dram:

            # ---- loads: x first (groupnorm is the critical path at startup) ----
            xt = big.tile([128, 4, 1024], f32)
            xr = xin.rearrange("(ci p) t -> p ci t", p=128)
            for ci in range(4):
                nc.sync.dma_start(out=xt[:, ci, :], in_=xr[:, ci, :])
            gam_t = const.tile([128, 4], f32)
            nc.sync.dma_start(out=gam_t, in_=gam.rearrange("(ci p) -> p ci", p=128))
            bet_t = const.tile([128, 4], f32)
            nc.sync.dma_start(out=bet_t, in_=bet.rearrange("(ci p) -> p ci", p=128))
            eps_t = const.tile([128, 1], f32)
            nc.vector.memset(eps_t, EPS)
            wqkT_t = const.tile([128, 4, 1024], f32r)
            wqr = wqkT.rearrange("(ci p) o -> p ci o", p=128)
            for ci in range(4):
                nc.sync.dma_start(out=wqkT_t[:, ci, :], in_=wqr[:, ci, :])
            wvT_t = const.tile([128, 4, 512], f32r)
            nc.sync.dma_start(out=wvT_t, in_=wvT.rearrange("(ci p) o -> p ci o", p=128))
            wpT_t = const.tile([128, 4, 512], f32r)
            nc.sync.dma_start(out=wpT_t, in_=wpT.rearrange("(ci p) o -> p ci o", p=128))
            bqk_t = const.tile([128, 8], f32)
            nc.sync.dma_start(out=bqk_t, in_=bqk.rearrange("(oi p) -> p oi", p=128))
            bv_b = const.tile([128, 512], f32)
            nc.sync.dma_start(out=bv_b, in_=bass.AP(
                tensor=bv, offset=0, ap=[[0, 128], [1, 512]]))
            bp_t = const.tile([128, 4], f32)
            nc.sync.dma_start(out=bp_t, in_=bp.rearrange("(ci p) -> p ci", p=128))

            # ---- GroupNorm ----
            # per-channel mean/var over t (bn_stats in 2 chunks of 512)
            chmv = gn.tile([128, 4, 2], f32)
            for ci in range(4):
                st = gn.tile([128, 2, 6], f32, tag="st")
                xv = xt[:, ci, :].rearrange("p (n f) -> p n f", f=512)
                for sub in range(2):
                    nc.vector.bn_stats(out=st[:, sub, :], in_=xv[:, sub, :])
                nc.vector.bn_aggr(out=chmv[:, ci, :], in_=st)
            # shuffle per-channel stats -> per-group rows [32, 16, 2]
            gst = gn.tile([32, 16, 2], f32)
            for ci in range(4):
                nc.sync.dma_start(out=gst[ci * 8:(ci + 1) * 8, :, :], in_=chmv[:, ci, :])
            # group mean / rstd  [32, 1]
            sm = gn.tile([32, 1], f32)
            nc.vector.tensor_reduce(out=sm, in_=gst[:, :, 0:1].rearrange("g j k -> g (j k)"),
                                    axis=mybir.AxisListType.X, op=Alu.add)
            s2 = gn.tile([32, 16], f32)
            nc.vector.tensor_mul(out=s2, in0=gst[:, :, 0:1].rearrange("g j k -> g (j k)"),
                                 in1=gst[:, :, 0:1].rearrange("g j k -> g (j k)"))
            nc.vector.tensor_add(out=s2, in0=s2, in1=gst[:, :, 1:2].rearrange("g j k -> g (j k)"))
            ss = gn.tile([32, 1], f32)
            nc.vector.tensor_reduce(out=ss, in_=s2, axis=mybir.AxisListType.X, op=Alu.add)
            grp = gn.tile([32, 2], f32)
            mg = grp[:, 0:1]
            nc.vector.tensor_scalar_mul(out=mg, in0=sm, scalar1=1.0 / GS)
            vg = grp[:, 1:2]
            nc.vector.tensor_scalar_mul(out=vg, in0=ss, scalar1=1.0 / GS)
            mg2 = gn.tile([32, 1], f32)
            nc.vector.tensor_mul(out=mg2, in0=mg, in1=mg)
            nc.vector.tensor_sub(out=vg, in0=vg, in1=mg2)
            # rstd = 1/sqrt(vg + eps)
            nc.scalar.activation(out=vg, in_=vg, func=Act.Sqrt, bias=eps_t[:32], scale=1.0)
            nc.vector.reciprocal(out=vg, in_=vg)
            # broadcast group stats back to channels via DRAM round-trip
            gd = dram.tile([32, 2], f32)
            nc.sync.dma_start(out=gd, in_=grp)
            chms = gn.tile([128, 4, 2], f32)
            for ci in range(4):
                src = bass.AP(tensor=gd.tensor, offset=gd.offset + ci * 8 * 2,
                              ap=[[2, 8], [0, 16], [1, 2]])
                nc.sync.dma_start(out=chms[:, ci, :], in_=src)
            # per-channel scale/shift  [128, 4]
            scl = gn.tile([128, 4], f32)
            nc.vector.tensor_mul(out=scl, in0=gam_t, in1=chms[:, :, 1])
            sht = gn.tile([128, 4], f32)
            nc.vector.tensor_mul(out=sht, in0=scl, in1=chms[:, :, 0])
            nc.vector.tensor_sub(out=sht, in0=bet_t, in1=sht)
            # h = x * scl + sht   (f32r, feeds matmuls)
            ht = big.tile([128, 4, 1024], f32r)
            for ci in range(4):
                nc.vector.tensor_scalar(out=ht[:, ci, :], in0=xt[:, ci, :],
                                        scalar1=scl[:, ci:ci + 1], scalar2=sht[:, ci:ci + 1],
                                        op0=Alu.mult, op1=Alu.add)

            # ---- q,k = WqkT.T @ h ----
            # o-blocks 0..3: q for head pairs (0,1)..(6,7); blocks 4..7: k likewise.
            # qk[pi] layout: [p, 2, t] with p<64 = head 2pi, p>=64 = head 2pi+1;
            # slot 0 = q, slot 1 = k  (so lhsT/rhs of the score matmul share a base
            # partition, and the two heads of a pair occupy different PE row groups).
            qk = [qkp.tile([128, 2, 1024], f32r, tag="qk", name=f"qk{i}") for i in range(4)]
            for oi in range(8):
                pi, side = oi % 4, oi // 4
                pqk = ps.tile([128, 1024], f32, tag="s")
                for ci in range(4):
                    for ni in range(2):
                        nc.tensor.matmul(pqk[:, ni * 512:(ni + 1) * 512],
                                         lhsT=wqkT_t[:, ci, oi * 128:(oi + 1) * 128],
                                         rhs=ht[:, ci, ni * 512:(ni + 1) * 512],
                                         start=(ci == 0), stop=(ci == 3))
                nc.vector.tensor_scalar_add(out=qk[pi][:, side, :], in0=pqk,
                                            scalar1=bqk_t[:, oi:oi + 1])

            # ---- vT[t, o_v] (+ ones col per head) ----
            vTa = big.tile([128, 8, 8, 65], f32r)  # [t_part, ti, hd, ch+1]
            for ti in range(8):
                nc.sync.dma_start(out=vTa[:, ti, :, 64:65].rearrange("p h k -> p (h k)"),
                                  in_=onesc[:, :])
                pv = ps.tile([128, 1024], f32, tag="s")
                for ci in range(4):
                    nc.tensor.matmul(pv[:, 0:512], lhsT=ht[:, ci, ti * 128:(ti + 1) * 128],
                                     rhs=wvT_t[:, ci, :], start=(ci == 0), stop=(ci == 3))
                nc.vector.tensor_add(
                    out=vTa[:, ti, :, 0:64],
                    in0=pv[:, 0:512].rearrange("p (h c) -> p h c", h=8),
                    in1=bv_b.rearrange("p (h c) -> p h c", h=8))

            # ---- attention, one head pair at a time ----
            at_ = big.tile([128, 4, 1024], f32r)  # a[c(hd-major), t]
            for pi in range(4):
                pa = [ps.tile([128, 1024], f32, tag="a", name=f"pa{pi}_{i}") for i in range(2)]
                for si in range(8):
                    pss = [ps.tile([128, 1024], f32, tag="s", name=f"pss{pi}_{si}_{i}") for i in range(2)]
                    # score matmuls for both heads: different PE row groups -> concurrent
                    for half in range(2):
                        base = 64 * half
                        for ni in range(2):
                            nc.tensor.matmul(
                                pss[half][:, ni * 512:(ni + 1) * 512],
                                lhsT=qk[pi][base:base + 64, 1, si * 128:(si + 1) * 128],
                                rhs=qk[pi][base:base + 64, 0, ni * 512:(ni + 1) * 512],
                                start=True, stop=True)
                    ess = []
                    for half in range(2):
                        es = esp.tile([128, 1024], f32r, tag="es")
                        nc.scalar.activation(out=es, in_=pss[half], func=Act.Exp)
                        ess.append(es)
                    for half in range(2):
                        hd = 2 * pi + half
                        for ni in range(2):
                            nc.tensor.matmul(pa[half][0:65, ni * 512:(ni + 1) * 512],
                                             lhsT=vTa[:, si, hd, :],
                                             rhs=ess[half][:, ni * 512:(ni + 1) * 512],
                                             start=(si == 0), stop=(si == 7))
                # softmax denominator: row 64 of pa.  Evacuate psum right away
                # (releases the "a" banks for the next pair), approx-reciprocal
                # the Z row, broadcast it to 64 partitions via a DRAM round
                # trip, and normalize during a second pass.
                for half in range(2):
                    hd = 2 * pi + half
                    au = zp.tile([65, 1024], f32, tag="au")
                    nc.vector.tensor_copy(out=au, in_=pa[half][0:65, :])
                    # reshape Z across 128 partitions (DRAM round trip), exact
                    # reciprocal on [128, 8] (cheap), reshape + broadcast back
                    zd = dram.tile([1, 1024], f32, tag="zd")
                    nc.sync.dma_start(out=zd, in_=au[64:65, :])
                    zq = zp.tile([128, 8], f32, tag="zq")
                    nc.sync.dma_start(out=zq, in_=bass.AP(
                        tensor=zd.tensor, offset=zd.offset, ap=[[8, 128], [1, 8]]))
                    nc.vector.reciprocal(out=zq, in_=zq)
                    zd2 = dram.tile([1, 1024], f32, tag="zd2")
                    nc.sync.dma_start(
                        out=bass.AP(tensor=zd2.tensor, offset=zd2.offset,
                                    ap=[[8, 128], [1, 8]]), in_=zq)
                    zb = zp.tile([64, 1024], f32, tag="zb")
                    nc.sync.dma_start(out=zb, in_=bass.AP(
                        tensor=zd2.tensor, offset=zd2.offset, ap=[[0, 64], [1, 1024]]))
                    nc.vector.tensor_tensor(
                        out=at_[(hd % 2) * 64:(hd % 2) * 64 + 64, hd // 2, :],
                        in0=au[0:64, :], in1=zb, op=Alu.mult)

            # ---- proj + bias + residual ----
            for oi in range(4):
                pp = ps.tile([128, 1024], f32, tag="s")
                for ci in range(4):
                    for ni in range(2):
                        nc.tensor.matmul(pp[:, ni * 512:(ni + 1) * 512],
                                         lhsT=wpT_t[:, ci, oi * 128:(oi + 1) * 128],
                                         rhs=at_[:, ci, ni * 512:(ni + 1) * 512],
                                         start=(ci == 0), stop=(ci == 3))
                ot = qkp.tile([128, 1024], f32, tag="qk")
                nc.vector.scalar_tensor_tensor(out=ot, in0=pp, scalar=bp_t[:, oi:oi + 1],
                                               in1=xt[:, oi, :], op0=Alu.add, op1=Alu.add)
                nc.sync.dma_start(
                    out=outd.rearrange("(ci p) t -> p ci t", p=128)[:, oi, :], in_=ot)

    _split_multi_waits(nc)
    return nc


def _prep_host(x, gamma, beta, w_qkv, b_qkv, w_proj, b_proj):
    """Host-side weight permutation/pre-scaling + per-core input maps."""
    x = np.ascontiguousarray(x, dtype=np.float32).reshape(B, C, T)
    scale2 = 1.0 / np.sqrt(CH)  # folded into q (exact: 0.125 is a power of two)

    w = np.asarray(w_qkv, dtype=np.float32).reshape(H, 3, CH, C)
    bq = np.asarray(b_qkv, dtype=np.float32).reshape(H, 3, CH)
    wq = w[:, 0] * scale2          # [hd, 64, c]
    wk = w[:, 1]
    wv = w[:, 2]
    # o-block order: 4 q-blocks (one per head pair: [q_{2i}; q_{2i+1}]), 4 k-blocks
    qcols = wq.reshape(4, 128, C).transpose(2, 0, 1)            # [c, pi, 128]
    kcols = wk.reshape(4, 128, C).transpose(2, 0, 1)
    wqkT_host = np.ascontiguousarray(
        np.concatenate([qcols, kcols], axis=1).reshape(C, 1024))
    bqk_host = np.ascontiguousarray(np.concatenate(
        [(bq[:, 0] * scale2).reshape(4, 128), bq[:, 1].reshape(4, 128)], axis=0
    ).reshape(1024))
    wvT_host = np.ascontiguousarray(wv.transpose(2, 0, 1).reshape(C, C))
    bv_host = np.ascontiguousarray(bq[:, 2].reshape(C))
    wpT_host = np.ascontiguousarray(np.asarray(w_proj, dtype=np.float32).T)
    ones_host = np.ones((128, 8), np.float32)

    common = {
        "wqkT": wqkT_host, "wvT": wvT_host, "wpT": wpT_host,
        "bqk": bqk_host, "bv": bv_host,
        "bp": np.ascontiguousarray(np.asarray(b_proj, np.float32)),
        "gam": np.ascontiguousarray(np.asarray(gamma, np.float32)),
        "bet": np.ascontiguousarray(np.asarray(beta, np.float32)),
        "onesc": ones_host,
    }
    return [dict(common, xin=np.ascontiguousarray(x[b])) for b in range(B)]


def kernel(x, gamma, beta, w_qkv, b_qkv, w_proj, b_proj):
    from concourse.bass_utils import run_bass_kernel_spmd

    if "nc" not in _CACHE:
        _CACHE["nc"] = _build_nc()
    nc = _CACHE["nc"]

    in_maps = _prep_host(x, gamma, beta, w_qkv, b_qkv, w_proj, b_proj)
    kwargs = {}
    if TRACE:
        _install_ntff_hook()
        kwargs["trace"] = True
    res = run_bass_kernel_spmd(nc, in_maps, core_ids=list(range(NCORES)), **kwargs)
    if TRACE:
        _CACHE["last_result"] = res
    out = np.stack([r["outd"] for r in res.results], axis=0)
    return out.reshape(B, C, HW, HW)


# revision 15
# speedup vs baseline: 1.3935x; 1.2198x over previous
"""Trainium2 Bass kernel for nn_AttentionBlock (GroupNorm + MHA + proj + residual).

Sharding: data-parallel over batch; 8 batches -> 8 NeuronCores, one batch each.

Per-core layout (c=512 channels, t=1024 spatial, H=8 heads, ch=64):
  - x, h kept as [c-on-partitions, t] (4 tiles of [128, 1024])
  - GroupNorm group-reduction done with two tiny mask matmuls on the PE
    (no cross-partition DMA shuffles)
  - q,k per head PAIR: qk[pi] = [p, {q,k}, t], partitions 0-63 = head 2pi,
    64-127 = head 2pi+1 (w_qkv rows permuted host-side; q pre-scaled by
    1/sqrt(ch) = 0.125, exact).  The two heads of a pair use different PE row
    groups, so their score matmuls run concurrently.
  - v computed directly transposed: vT[t, o_v] via matmul(lhsT=h, rhs=WvT), with
    a ones column per head -> the AV matmul also emits the softmax denominator Z
  - scores computed transposed: S^T[s, t] = k^T q, so exp(S^T) (ACT, psum->sbuf)
    feeds the AV matmul with s as the contraction dim; softmax skips the
    max-subtraction (scores are ~N(0,1), exp is safe in fp32)
  - Z normalization: evacuate AV psum, reshape Z across 128 partitions via a
    DRAM round trip, exact reciprocal on [128, 8], broadcast back, fused into
    the normalization multiply
  - proj + bias + residual fused in one scalar_tensor_tensor per output tile
All matmuls run in fp32r (same PE rate as bf16 here; ~1.5e-4 max rel err).
"""

import numpy as np

B, C, HW, T = 8, 512, 32, 1024
H, CH = 8, 64
G, GS = 32, 16
EPS = 1e-5
NCORES = 8

_CACHE = {}
TRACE = False  # test harness can set kernel.TRACE = True to get a profile


def _install_ntff_hook():
    import sys, types
    if 'antenv.axon_hooks' in sys.modules:
        return
    mod = types.ModuleType('antenv.axon_hooks')
    state = {'hook': None}
    mod.set_axon_ntff_profile_hook = lambda h: state.__setitem__('hook', h)
    mod.get_axon_ntff_profile_hook = lambda: state['hook']
    sys.modules['antenv.axon_hooks'] = mod
    import antenv
    antenv.axon_hooks = mod
    try:
        from trn_agent_boot.trn_boot import _ntff_profile_via_ctypes
        mod.set_axon_ntff_profile_hook(_ntff_profile_via_ctypes('/opt/axon/libaxon_pjrt.so'))
    except Exception:
        pass


def _patch_ldw_opt():
    """Let walrus dedup back-to-back LDWEIGHTS of the same stationary operand."""
    import concourse.bass_utils as bu
    if getattr(bu, "_ldw_opt_patched", False):
        return
    orig = bu.run_command

    def patched(argv, **kw):
        argv = ["--enable-ldw-opt=true" if a == "--enable-ldw-opt=false" else a
                for a in argv]
        return orig(argv, **kw)

    bu.run_command = patched
    bu._ldw_opt_patched = True


def _split_multi_waits(nc, max_waits=1):
    """This container's walrus supports only one sync wait per instruction; move
    extra waits onto same-engine no-ops inserted just before the instruction."""
    import concourse.mybir as mybir
    for f in nc.m.functions:
        for bb in f.blocks:
            insts = bb.instructions
            out = []
            changed = False
            for inst in insts:
                si = inst.sync_info
                waits = list(si.on_wait) if si is not None and si.on_wait else []
                if len(waits) > max_waits:
                    changed = True
                    for j, w in enumerate(waits[:-max_waits]):
                        out.append(mybir.InstNoOp(
                            name=f"{inst.name}-ws{j}",
                            sync_info=mybir.SyncInfo(on_wait=[w], on_update=[]),
                            bass_nofuse=True,
                            engine=inst.engine,
                        ))
                    inst.sync_info = mybir.SyncInfo(
                        on_wait=waits[-max_waits:],
                        on_update=list(si.on_update) if si.on_update else [],
                    )
                out.append(inst)
            if changed:
                bb.instructions = out


def _build_nc():
    import concourse.bass as bass
    import concourse.tile as tile
    import concourse.mybir as mybir

    f32 = mybir.dt.float32
    f32r = mybir.dt.float32r
    Alu = mybir.AluOpType
    Act = mybir.ActivationFunctionType

    nc = bass.Bass()

    xin = nc.dram_tensor("xin", [C, T], f32, kind="ExternalInput")
    wqkT = nc.dram_tensor("wqkT", [C, 1024], f32r, kind="ExternalInput")
    wvT = nc.dram_tensor("wvT", [C, C], f32r, kind="ExternalInput")
    wpT = nc.dram_tensor("wpT", [C, C], f32r, kind="ExternalInput")
    bqk = nc.dram_tensor("bqk", [1024], f32, kind="ExternalInput")
    bv = nc.dram_tensor("bv", [C], f32, kind="ExternalInput")
    bp = nc.dram_tensor("bp", [C], f32, kind="ExternalInput")
    gam = nc.dram_tensor("gam", [C], f32, kind="ExternalInput")
    bet = nc.dram_tensor("bet", [C], f32, kind="ExternalInput")
    onesc = nc.dram_tensor("onesc", [128, 8], f32r, kind="ExternalInput")
    maskA = nc.dram_tensor("maskA", [128, 8], f32r, kind="ExternalInput")
    maskB = nc.dram_tensor("maskB", [8, 128], f32r, kind="ExternalInput")
    outd = nc.dram_tensor("outd", [C, T], f32, kind="ExternalOutput")

    with tile.TileContext(nc) as tc:
        with tc.tile_pool(name="const", bufs=1) as const, \
             tc.tile_pool(name="big", bufs=1) as big, \
             tc.tile_pool(name="qkp", bufs=4) as qkp, \
             tc.tile_pool(name="esp", bufs=8) as esp, \
             tc.tile_pool(name="zp", bufs=2) as zp, \
             tc.tile_pool(name="gn", bufs=2) as gn, \
             tc.tile_pool(name="ps", bufs=2, space="PSUM") as ps, \
             tc.tile_pool(name="dram", bufs=2, space="DRAM") as dram:

            # ---- loads.  x + groupnorm consts on the Sync queue (critical
            # path); weights and the rest issued from the GpSimd queue so they
            # don't delay groupnorm. ----
            xt = [big.tile([128, 1024], f32, tag=f"x{ci}", name=f"x{ci}") for ci in range(4)]
            xr = xin.rearrange("(ci p) t -> p ci t", p=128)
            for ci in range(4):
                nc.sync.dma_start(out=xt[ci], in_=xr[:, ci, :])
            gam_t = const.tile([128, 4], f32)
            nc.sync.dma_start(out=gam_t, in_=gam.rearrange("(ci p) -> p ci", p=128))
            bet_t = const.tile([128, 4], f32)
            nc.sync.dma_start(out=bet_t, in_=bet.rearrange("(ci p) -> p ci", p=128))
            mA = const.tile([128, 8], f32r)
            nc.sync.dma_start(out=mA, in_=maskA[:, :])
            mB = const.tile([8, 128], f32r)
            nc.sync.dma_start(out=mB, in_=maskB[:, :])
            eps_t = const.tile([128, 1], f32)
            nc.vector.memset(eps_t, EPS)

            wqkT_t = const.tile([128, 4, 1024], f32r)
            nc.gpsimd.dma_start(out=wqkT_t, in_=wqkT.rearrange("(ci p) o -> p ci o", p=128))
            wvT_t = const.tile([128, 4, 512], f32r)
            nc.gpsimd.dma_start(out=wvT_t, in_=wvT.rearrange("(ci p) o -> p ci o", p=128))
            wpT_t = const.tile([128, 4, 512], f32r)
            nc.gpsimd.dma_start(out=wpT_t, in_=wpT.rearrange("(ci p) o -> p ci o", p=128))
            bqk_t = const.tile([128, 8], f32)
            nc.gpsimd.dma_start(out=bqk_t, in_=bqk.rearrange("(oi p) -> p oi", p=128))
            bv_b = const.tile([128, 512], f32)
            nc.gpsimd.dma_start(out=bv_b, in_=bass.AP(
                tensor=bv, offset=0, ap=[[0, 128], [1, 512]]))
            bp_t = const.tile([128, 4], f32)
            nc.gpsimd.dma_start(out=bp_t, in_=bp.rearrange("(ci p) -> p ci", p=128))

            # ---- GroupNorm ----
            # per-channel mean/var over t (bn_stats in 2 chunks of 512)
            chmv = gn.tile([128, 4, 2], f32)
            for ci in range(4):
                st = gn.tile([128, 2, 6], f32, tag="st")
                xv = xt[ci].rearrange("p (n f) -> p n f", f=512)
                for sub in range(2):
                    nc.vector.bn_stats(out=st[:, sub, :], in_=xv[:, sub, :])
                nc.vector.bn_aggr(out=chmv[:, ci, :], in_=st)
            # per-channel [mean, var+mean^2] as f32r for the mask matmul
            s2ch = gn.tile([128, 4, 2], f32r)
            nc.vector.tensor_copy(out=s2ch[:, :, 0], in_=chmv[:, :, 0])
            t1 = gn.tile([128, 4], f32)
            nc.vector.tensor_mul(out=t1, in0=chmv[:, :, 0], in1=chmv[:, :, 0])
            nc.vector.tensor_add(out=s2ch[:, :, 1], in0=t1, in1=chmv[:, :, 1])
            # group sums: [8 groups-in-tile, (ci, k)]
            pg = ps.tile([128, 1024], f32, tag="s", name="pgn")
            nc.tensor.matmul(pg[0:8, 0:8], lhsT=mA,
                             rhs=s2ch.rearrange("p a b -> p (a b)"),
                             start=True, stop=True)
            gf = gn.tile([8, 4, 2], f32r)
            mg = gn.tile([8, 4], f32)
            nc.vector.tensor_scalar_mul(out=mg, in0=pg[0:8, 0:8].rearrange(
                "g (a b) -> g a b", b=2)[:, :, 0], scalar1=1.0 / GS)
            vg = gn.tile([8, 4], f32)
            nc.vector.tensor_scalar_mul(out=vg, in0=pg[0:8, 0:8].rearrange(
                "g (a b) -> g a b", b=2)[:, :, 1], scalar1=1.0 / GS)
            m2 = gn.tile([8, 4], f32)
            nc.vector.tensor_mul(out=m2, in0=mg, in1=mg)
            nc.vector.tensor_sub(out=vg, in0=vg, in1=m2)
            # rstd = 1/sqrt(vg + eps)
            nc.scalar.activation(out=vg, in_=vg, func=Act.Sqrt, bias=eps_t[:8], scale=1.0)
            nc.vector.reciprocal(out=vg, in_=vg)
            nc.vector.tensor_copy(out=gf[:, :, 0], in_=mg)
            nc.vector.tensor_copy(out=gf[:, :, 1], in_=vg)
            # broadcast group stats back to channels: [128, (ci, k)]
            pc = ps.tile([128, 1024], f32, tag="s", name="pgc")
            nc.tensor.matmul(pc[:, 0:8], lhsT=mB,
                             rhs=gf.rearrange("g a b -> g (a b)"),
                             start=True, stop=True)
            chms = pc[:, 0:8].rearrange("p (a b) -> p a b", b=2)
            # per-channel scale/shift  [128, 4]
            scl = gn.tile([128, 4], f32)
            nc.vector.tensor_mul(out=scl, in0=gam_t, in1=chms[:, :, 1])
            sht = gn.tile([128, 4], f32)
            nc.vector.tensor_mul(out=sht, in0=scl, in1=chms[:, :, 0])
            nc.vector.tensor_sub(out=sht, in0=bet_t, in1=sht)
            # h = x * scl + sht   (f32r, feeds matmuls)
            ht = [big.tile([128, 1024], f32r, tag=f"h{ci}", name=f"h{ci}") for ci in range(4)]
            for ci in range(4):
                nc.vector.tensor_scalar(out=ht[ci], in0=xt[ci],
                                        scalar1=scl[:, ci:ci + 1], scalar2=sht[:, ci:ci + 1],
                                        op0=Alu.mult, op1=Alu.add)

            # ---- q,k = WqkT.T @ h ----
            # o-blocks 0..3: q for head pairs (0,1)..(6,7); blocks 4..7: k.
            # Emit in order q0,k0,q1,k1,... so pair 0's attention can start early.
            qk = [qkp.tile([128, 2, 1024], f32r, tag="qk", name=f"qk{i}") for i in range(4)]
            for pi in range(4):
                for side in range(2):
                    oi = side * 4 + pi
                    pqk = ps.tile([128, 1024], f32, tag="s", name=f"pqk{oi}")
                    for ci in range(4):
                        for ni in range(2):
                            nc.tensor.matmul(pqk[:, ni * 512:(ni + 1) * 512],
                                             lhsT=wqkT_t[:, ci, oi * 128:(oi + 1) * 128],
                                             rhs=ht[ci][:, ni * 512:(ni + 1) * 512],
                                             start=(ci == 0), stop=(ci == 3))
                    nc.vector.tensor_scalar_add(out=qk[pi][:, side, :], in0=pqk,
                                                scalar1=bqk_t[:, oi:oi + 1])

            # ---- vT[t, o_v] (+ ones col per head) ----
            vTa = big.tile([128, 8, 8, 65], f32r)  # [t_part, ti, hd, ch+1]
            for ti in range(8):
                nc.gpsimd.dma_start(out=vTa[:, ti, :, 64:65].rearrange("p h k -> p (h k)"),
                                    in_=onesc[:, :])
                pv = ps.tile([128, 1024], f32, tag="s", name=f"pv{ti}")
                for ci in range(4):
                    nc.tensor.matmul(pv[:, 0:512], lhsT=ht[ci][:, ti * 128:(ti + 1) * 128],
                                     rhs=wvT_t[:, ci, :], start=(ci == 0), stop=(ci == 3))
                nc.vector.tensor_add(
                    out=vTa[:, ti, :, 0:64],
                    in0=pv[:, 0:512].rearrange("p (h c) -> p h c", h=8),
                    in1=bv_b.rearrange("p (h c) -> p h c", h=8))

            # ---- attention, one head pair at a time ----
            at_ = [big.tile([128, 1024], f32r, tag=f"at{pi}", name=f"at{pi}")
                   for pi in range(4)]  # a[c(hd-major), t] per pair
            for pi in range(4):
                pa = [ps.tile([128, 1024], f32, tag="a", name=f"pa{pi}_{i}") for i in range(2)]
                for si in range(8):
                    pss = [ps.tile([128, 1024], f32, tag="s", name=f"pss{pi}_{si}_{i}")
                           for i in range(2)]
                    # score matmuls for both heads: different PE row groups
                    for half in range(2):
                        base = 64 * half
                        for ni in range(2):
                            nc.tensor.matmul(
                                pss[half][:, ni * 512:(ni + 1) * 512],
                                lhsT=qk[pi][base:base + 64, 1, si * 128:(si + 1) * 128],
                                rhs=qk[pi][base:base + 64, 0, ni * 512:(ni + 1) * 512],
                                start=True, stop=True)
                    ess = []
                    for half in range(2):
                        es = esp.tile([128, 1024], f32r, tag="es")
                        nc.scalar.activation(out=es, in_=pss[half], func=Act.Exp)
                        ess.append(es)
                    for half in range(2):
                        hd = 2 * pi + half
                        for ni in range(2):
                            nc.tensor.matmul(pa[half][0:65, ni * 512:(ni + 1) * 512],
                                             lhsT=vTa[:, si, hd, :],
                                             rhs=ess[half][:, ni * 512:(ni + 1) * 512],
                                             start=(si == 0), stop=(si == 7))
                # normalize: evacuate, reciprocal of Z on [128, 8], broadcast
                for half in range(2):
                    hd = 2 * pi + half
                    au = zp.tile([65, 1024], f32, tag="au")
                    nc.vector.tensor_copy(out=au, in_=pa[half][0:65, :])
                    zd = dram.tile([1, 1024], f32, tag="zd")
                    nc.sync.dma_start(out=zd, in_=au[64:65, :])
                    zq = zp.tile([128, 8], f32, tag="zq")
                    nc.sync.dma_start(out=zq, in_=bass.AP(
                        tensor=zd.tensor, offset=zd.offset, ap=[[8, 128], [1, 8]]))
                    nc.vector.reciprocal(out=zq, in_=zq)
                    zd2 = dram.tile([1, 1024], f32, tag="zd2")
                    nc.sync.dma_start(
                        out=bass.AP(tensor=zd2.tensor, offset=zd2.offset,
                                    ap=[[8, 128], [1, 8]]), in_=zq)
                    zb = zp.tile([64, 1024], f32, tag="zb")
                    nc.sync.dma_start(out=zb, in_=bass.AP(
                        tensor=zd2.tensor, offset=zd2.offset, ap=[[0, 64], [1, 1024]]))
                    nc.vector.tensor_tensor(
                        out=at_[pi][(hd % 2) * 64:(hd % 2) * 64 + 64, :],
                        in0=au[0:64, :], in1=zb, op=Alu.mult)

            # ---- proj + bias + residual ----
            for oi in range(4):
                pp = ps.tile([128, 1024], f32, tag="s", name=f"pp{oi}")
                for ci in range(4):
                    for ni in range(2):
                        nc.tensor.matmul(pp[:, ni * 512:(ni + 1) * 512],
                                         lhsT=wpT_t[:, ci, oi * 128:(oi + 1) * 128],
                                         rhs=at_[ci][:, ni * 512:(ni + 1) * 512],
                                         start=(ci == 0), stop=(ci == 3))
                ot = qkp.tile([128, 1024], f32, tag="qk", name=f"ot{oi}")
                nc.vector.scalar_tensor_tensor(out=ot, in0=pp, scalar=bp_t[:, oi:oi + 1],
                                               in1=xt[oi], op0=Alu.add, op1=Alu.add)
                nc.sync.dma_start(
                    out=outd.rearrange("(ci p) t -> p ci t", p=128)[:, oi, :], in_=ot)

    _split_multi_waits(nc)
    return nc


def _prep_host(x, gamma, beta, w_qkv, b_qkv, w_proj, b_proj):
    """Host-side weight permutation/pre-scaling + per-core input maps."""
    x = np.ascontiguousarray(x, dtype=np.float32).reshape(B, C, T)
    scale2 = 1.0 / np.sqrt(CH)  # folded into q (exact: 0.125 is a power of two)

    w = np.asarray(w_qkv, dtype=np.float32).reshape(H, 3, CH, C)
    bq = np.asarray(b_qkv, dtype=np.float32).reshape(H, 3, CH)
    wq = w[:, 0] * scale2          # [hd, 64, c]
    wk = w[:, 1]
    wv = w[:, 2]
    # o-block order: 4 q-blocks (one per head pair: [q_{2i}; q_{2i+1}]), 4 k-blocks
    qcols = wq.reshape(4, 128, C).transpose(2, 0, 1)            # [c, pi, 128]
    kcols = wk.reshape(4, 128, C).transpose(2, 0, 1)
    wqkT_host = np.ascontiguousarray(
        np.concatenate([qcols, kcols], axis=1).reshape(C, 1024))
    bqk_host = np.ascontiguousarray(np.concatenate(
        [(bq[:, 0] * scale2).reshape(4, 128), bq[:, 1].reshape(4, 128)], axis=0
    ).reshape(1024))
    wvT_host = np.ascontiguousarray(wv.transpose(2, 0, 1).reshape(C, C))
    bv_host = np.ascontiguousarray(bq[:, 2].reshape(C))
    wpT_host = np.ascontiguousarray(np.asarray(w_proj, dtype=np.float32).T)
    ones_host = np.ones((128, 8), np.float32)
    maskA_host = np.zeros((128, 8), np.float32)
    for p in range(128):
        maskA_host[p, p // 16] = 1.0
    maskB_host = np.ascontiguousarray(maskA_host.T)

    common = {
        "wqkT": wqkT_host, "wvT": wvT_host, "wpT": wpT_host,
        "bqk": bqk_host, "bv": bv_host,
        "bp": np.ascontiguousarray(np.asarray(b_proj, np.float32)),
        "gam": np.ascontiguousarray(np.asarray(gamma, np.float32)),
        "bet": np.ascontiguousarray(np.asarray(beta, np.float32)),
        "onesc": ones_host, "maskA": maskA_host, "maskB": maskB_host,
    }
    return [dict(common, xin=np.ascontiguousarray(x[b])) for b in range(B)]


def kernel(x, gamma, beta, w_qkv, b_qkv, w_proj, b_proj):
    from concourse.bass_utils import run_bass_kernel_spmd

    # _patch_ldw_opt()  # caused a runtime deadlock; see notes
    if "nc" not in _CACHE:
        _CACHE["nc"] = _build_nc()
    nc = _CACHE["nc"]

    in_maps = _prep_host(x, gamma, beta, w_qkv, b_qkv, w_proj, b_proj)
    kwargs = {}
    if TRACE:
        _install_ntff_hook()
        kwargs["trace"] = True
    res = run_bass_kernel_spmd(nc, in_maps, core_ids=list(range(NCORES)), **kwargs)
    if TRACE:
        _CACHE["last_result"] = res
    out = np.stack([r["outd"] for r in res.results], axis=0)
    return out.reshape(B, C, HW, HW)
